# revision 1
# baseline (speedup 1.0000x reference)
"""Multi-head causal attention (B=8, T=2048, C=384, H=6, Dh=64) on 8 TRN2 cores.

Sharding: data-parallel over batch — core b computes batch element b end to end
(no collectives).

Per-core kernel layout (all "T" means transposed, head-dim/channel on
partitions):
  xT   [128, 3, 2048]  bf16   c = 128*ci + p
  wq/wk[128, 3, 384]   bf16   packed Wq[h,c,d] -> [c, h*64+d]
  wv   [128, 3, 384]   bf16
  wp   [128, 3, 384]   bf16   Wp[c, e] -> [128, ci, e]
  mask [128, 384]      f32    mask[p, g] = 0 if p <= g-128 else -1e30
  bp   [1, 384]        f32r   bias row (K=1 matmul into output PSUM)

Compute per core:
  QT/KT [hd, t] via matmul(lhsT=w chunk, rhs=xT)      (hd = h*64+d, 3 blocks)
  V_aug [s, 65] per (s-chunk, head), last col = 1     (stationary for PV)
  per q-block j (256 wide), head h:
    ST chunks [s=128, t=256] = KT^T-slice @ QT-slice  (K = d = 64)
    causal mask add on diagonal chunk, exp (ACT, scale=Dh^-0.5) -> P bf16
    O_aug [65, 256] += V_aug^T @ P                    (row 64 = softmax denom)
    recip = 1/denom; B = ones64^T @ recip (K=1)       (broadcast over d)
    attT [hd, t] slice = O[0:64] * B                  (DVE, bf16)
  out [t, e] = attT^T-slice @ wp + ones128^T @ bp     (K = hd, 3 chunks + bias)
"""

import numpy as np
import ml_dtypes

import concourse.bass as bass
import concourse.tile as tile
from concourse import bacc, mybir
from concourse.bass import ts, ds

F32 = mybir.dt.float32
F32R = mybir.dt.float32r
BF16 = mybir.dt.bfloat16
AF = mybir.ActivationFunctionType

B, T, C = 8, 2048, 384
H, DH = 6, 64
SCALE = DH ** -0.5
NEG = -1e30
NCORES = 8
TJ = 512            # q-block width
NJ = T // TJ        # 8 q-blocks
SC = 128            # s-chunk
NCI = C // 128      # 3 channel chunks


def build_kernel():
    nc = bacc.Bacc("TRN2", target_bir_lowering=False, debug=False)

    xT_d = nc.dram_tensor("xT", [128, NCI, T], BF16, kind="ExternalInput").ap()
    wq_d = nc.dram_tensor("wq", [128, NCI, C], BF16, kind="ExternalInput").ap()
    wk_d = nc.dram_tensor("wk", [128, NCI, C], BF16, kind="ExternalInput").ap()
    wv_d = nc.dram_tensor("wv", [128, NCI, C], BF16, kind="ExternalInput").ap()
    wp_d = nc.dram_tensor("wp", [128, NCI, C], BF16, kind="ExternalInput").ap()
    mask_d = nc.dram_tensor("mask", [128, 128], F32, kind="ExternalInput").ap()
    biasb_d = nc.dram_tensor("biasb", [128, 384], F32, kind="ExternalInput").ap()
    iden_d = nc.dram_tensor("iden", [128, 128], F32, kind="ExternalInput").ap()
    y_d = nc.dram_tensor("y", [T, C], F32, kind="ExternalOutput").ap()

    with tile.TileContext(nc) as tc:
        with tc.tile_pool(name="const", bufs=1) as cpool:
            xT = cpool.tile([128, NCI, T], BF16)
            wq = cpool.tile([128, NCI, C], BF16)
            wk = cpool.tile([128, NCI, C], BF16)
            wv = cpool.tile([128, NCI, C], BF16)
            wp = cpool.tile([128, NCI, C], BF16)
            mask = cpool.tile([128, 128], F32)
            biasb = cpool.tile([128, 384], F32)
            iden = cpool.tile([128, 128], F32)
            QT = cpool.tile([128, NCI, T], BF16)
            KT = cpool.tile([128, NCI, T], BF16)
            attT = cpool.tile([128, NCI, T], BF16)
            Vt = cpool.tile([128, 16, H, 65], BF16)

            for ci in range(NCI):
                nc.sync.dma_start(xT[:, ci, :], xT_d[:, ci, :])
            nc.sync.dma_start(wq[:], wq_d[:])
            nc.sync.dma_start(wk[:], wk_d[:])
            nc.sync.dma_start(wv[:], wv_d[:])
            nc.sync.dma_start(wp[:], wp_d[:])
            nc.sync.dma_start(mask[:], mask_d[:])
            nc.sync.dma_start(biasb[:], biasb_d[:])
            nc.sync.dma_start(iden[:], iden_d[:])
            # whole-tile memset (contiguous; strided memset fails ISA check);
            # V copies below overwrite cols 0:64, leaving col 64 == 1.0
            nc.gpsimd.memset(Vt[:], 1.0)

            # ---- phase 1: projections ----
            with tc.tile_pool(name="pqk", bufs=2, space="PSUM") as pqk, \
                 tc.tile_pool(name="pv", bufs=2, space="PSUM") as pvp:
                for dst, w in ((QT, wq), (KT, wk)):
                    for pi in range(NCI):
                        for tcn in range(T // 512):
                            ps = pqk.tile([128, 512], F32, tag="pqk")
                            for ci in range(NCI):
                                nc.tensor.matmul(
                                    ps[:],
                                    lhsT=w[:, ci, ts(pi, 128)],
                                    rhs=xT[:, ci, ts(tcn, 512)],
                                    start=(ci == 0), stop=(ci == NCI - 1),
                                )
                            nc.vector.tensor_copy(dst[:, pi, ts(tcn, 512)], ps[:])
                for si in range(16):
                    ps = pvp.tile([128, C], F32, tag="pv")
                    for ci in range(NCI):
                        nc.tensor.matmul(
                            ps[:],
                            lhsT=xT[:, ci, ts(si, 128)],
                            rhs=wv[:, ci, :],
                            start=(ci == 0), stop=(ci == NCI - 1),
                        )
                    nc.vector.tensor_copy(
                        Vt[:, si, :, 0:64],
                        ps[:].rearrange("p (h d) -> p h d", h=H),
                    )

            # ---- phase 2+3: attention + output projection ----
            with tc.tile_pool(name="sps", bufs=2, space="PSUM") as sps, \
                 tc.tile_pool(name="ops", bufs=2, space="PSUM") as ops, \
                 tc.tile_pool(name="dps", bufs=2, space="PSUM") as dps, \
                 tc.tile_pool(name="ups", bufs=2, space="PSUM") as ups, \
                 tc.tile_pool(name="pp", bufs=4) as pp, \
                 tc.tile_pool(name="rp", bufs=2) as rp, \
                 tc.tile_pool(name="yp", bufs=2) as yp:
                NCH = TJ // SC  # s-chunks per q-block (4)
                for j in range(NJ):
                    # denominators of all 6 heads, transposed: dT[t%128, h*4+q]
                    dT = dps.tile([128, NCH * H], F32, tag="dT")
                    for h in range(H):
                        po = (h % 2) * 64     # partition offset inside hd-block
                        bi = h // 2           # hd block index
                        O = ops.tile([65, TJ], F32, tag="O")
                        for i in range(NCH * j + NCH):
                            fringe = i >= NCH * j
                            d = SC * i - TJ * j if fringe else 0
                            S = sps.tile([128, TJ], F32, tag="S")
                            nc.tensor.matmul(
                                S[:, d:TJ],
                                lhsT=KT[po:po + 64, bi, ts(i, SC)],
                                rhs=QT[po:po + 64, bi, ds(j * TJ + d, TJ - d)],
                                start=True, stop=True,
                            )
                            P = pp.tile([128, TJ], BF16, tag="P")
                            nc.scalar.activation(P[:, d:TJ], S[:, d:TJ],
                                                 AF.Exp, scale=SCALE)
                            if fringe:
                                if d > 0:
                                    nc.gpsimd.memset(P[:, 0:d], 0.0)
                                # diagonal window [d, d+128): keep iff p <= f-d
                                nc.gpsimd.affine_select(
                                    out=P[:, d:d + 128], in_=P[:, d:d + 128],
                                    pattern=[[1, 128]],
                                    compare_op=mybir.AluOpType.is_ge,
                                    fill=0.0, base=0, channel_multiplier=-1,
                                )
                            nc.tensor.matmul(
                                O[:],
                                lhsT=Vt[:, i, h, :],
                                rhs=P[:],
                                start=(i == 0), stop=(i == NCH * j + NCH - 1),
                            )
                        # stage unnormalized attT (bf16) and transposed denom
                        nc.vector.tensor_copy(
                            attT[po:po + 64, bi, ts(j, TJ)], O[0:64, :]
                        )
                        dsb = rp.tile([1, TJ], F32, tag="dsb")
                        nc.vector.tensor_copy(dsb[:], O[64:65, :])
                        for q in range(NCH):
                            nc.tensor.transpose(
                                dT[:, h * NCH + q:h * NCH + q + 1],
                                dsb[0:1, ts(q, 128)], iden[0:1, 0:1],
                            )
                    rT = rp.tile([128, NCH * H], F32, tag="rT")
                    nc.vector.reciprocal(rT[:], dT[:])
                    # ---- per-head output projection, normalized via stt ----
                    for q in range(NCH):
                        tb = NCH * j + q
                        Y = yp.tile([128, C], F32, tag="Y")
                        for h in range(H):
                            po = (h % 2) * 64
                            bi = h // 2
                            U = ups.tile([128, C], F32, tag="U")
                            nc.tensor.matmul(
                                U[:],
                                lhsT=attT[po:po + 64, bi, ts(tb, 128)],
                                rhs=wp[po:po + 64, bi, :],
                                start=True, stop=True,
                            )
                            sc = rT[:, h * NCH + q:h * NCH + q + 1]
                            nc.vector.scalar_tensor_tensor(
                                out=Y[:], in0=U[:], scalar=sc,
                                in1=(biasb[:] if h == 0 else Y[:]),
                                op0=mybir.AluOpType.mult,
                                op1=mybir.AluOpType.add,
                            )
                        nc.sync.dma_start(y_d[ts(tb, 128), :], Y[:])

    nc.compile()
    return nc


def _prep_inputs(x, Wq, Wk, Wv, Wp, bp):
    """Host-side shard + layout prep. Returns per-core input maps."""
    bf = ml_dtypes.bfloat16
    x = np.asarray(x, dtype=np.float32)

    def pack_w(W):  # [H, C, Dh] -> [128, NCI, H*Dh]
        Whd = np.transpose(np.asarray(W, np.float32), (1, 0, 2)).reshape(C, H * DH)
        return np.ascontiguousarray(
            Whd.reshape(NCI, 128, H * DH).transpose(1, 0, 2)
        ).astype(bf)

    wq_p, wk_p, wv_p = pack_w(Wq), pack_w(Wk), pack_w(Wv)
    wp_p = np.ascontiguousarray(
        np.asarray(Wp, np.float32).reshape(NCI, 128, C).transpose(1, 0, 2)
    ).astype(bf)

    f = np.arange(128)[None, :]
    p = np.arange(128)[:, None]
    mask = np.where(p <= f, 0.0, NEG).astype(np.float32)
    biasb = np.broadcast_to(np.asarray(bp, np.float32), (128, C)).copy()
    iden_np = np.eye(128, dtype=np.float32)

    in_maps = []
    for b in range(B):
        xT = np.ascontiguousarray(
            x[b].T.reshape(NCI, 128, T).transpose(1, 0, 2)
        ).astype(bf)
        in_maps.append({
            "xT": xT, "wq": wq_p, "wk": wk_p, "wv": wv_p, "wp": wp_p,
            "mask": mask, "biasb": biasb, "iden": iden_np,
        })
    return in_maps


_CACHE = {}


def kernel(x, Wq, Wk, Wv, Wp, bp):
    from concourse.bass_utils import run_bass_kernel_spmd

    if "nc" not in _CACHE:
        _CACHE["nc"] = build_kernel()
    nc = _CACHE["nc"]
    in_maps = _prep_inputs(x, Wq, Wk, Wv, Wp, bp)
    res = run_bass_kernel_spmd(nc, in_maps, list(range(NCORES)))
    out = np.stack([res.results[b]["y"] for b in range(B)], axis=0)
    return out.astype(np.float32)



# revision 5
# speedup vs baseline: 1.3604x; 1.3604x over previous
"""Multi-head causal attention (B=8, T=2048, C=384, H=6, Dh=64) on 8 TRN2 cores.

Sharding: data-parallel over batch - core b computes batch element b end to end
(no collectives).

v2 layout (all "T" means transposed, head-dim/channel on partitions):
  xT   [128, 3, 2048]  bf16   c = 128*ci + p
  wq/wk[128, 3, 384]   bf16   packed Wq[h,c,d] -> [c, h*64+d]
  wv/wp[128, 3, 384]   bf16
  biasb[128, 384]      f32    bias replicated across partitions

Per-core compute:
  QT/KT [hd, t] via matmul; Vt [s, h, 65] augmented (col 64 == 1 -> denom row).
  Attention per (j q-block of 512, hp head-pair, i s-chunk of 128):
    S-pair [128, 1024] = two concurrent K=64 matmuls (tile rows 0:64 / 64:128)
    one wide exp (ACT) -> P bf16 [128, 1024]; causal diag via affine_select
    PV per head accumulates O[65, 512] (row 64 = softmax denominator)
  Denominator path: denom rows -> reciprocal_approx_fast -> fp16 -> K=1
    broadcast matmuls -> recipB [128, 512] psum -> one tensor_mul normalizes
    attT in place.  Output projection is then a dense K=128 accumulation
    Y[t,e] = attT^T @ wp (+bias via tensor_add on eviction).
  Phase-1 projection work is drip-fed into the attention loop (4 tasks per
  head-pair boundary) to keep PE dense (HAM warm) while ACT streams exps.
"""

import numpy as np
import ml_dtypes

import concourse.bass as bass
import concourse.tile as tile
from concourse import bacc, mybir
from concourse.bass import ts, ds

F32 = mybir.dt.float32
BF16 = mybir.dt.bfloat16
FP16 = mybir.dt.float16
AF = mybir.ActivationFunctionType

B, T, C = 8, 2048, 384
H, DH = 6, 64
SCALE = DH ** -0.5
NCORES = 8
TJ = 512            # q-block width
NJ = T // TJ        # 4 q-blocks
SC = 128            # s-chunk
NCI = C // 128      # 3 channel chunks
NHP = H // 2        # 3 head pairs (= hd blocks)


def build_kernel():
    nc = bacc.Bacc("TRN2", target_bir_lowering=False, debug=False)

    xT_d = nc.dram_tensor("xT", [128, NCI, T], BF16, kind="ExternalInput").ap()
    wq_d = nc.dram_tensor("wq", [128, NCI, C], BF16, kind="ExternalInput").ap()
    wk_d = nc.dram_tensor("wk", [128, NCI, C], BF16, kind="ExternalInput").ap()
    wv_d = nc.dram_tensor("wv", [128, NCI, C], BF16, kind="ExternalInput").ap()
    wp_d = nc.dram_tensor("wp", [128, NCI, C], BF16, kind="ExternalInput").ap()
    biasb_d = nc.dram_tensor("biasb", [128, C], F32, kind="ExternalInput").ap()
    y_d = nc.dram_tensor("y", [T, C], F32, kind="ExternalOutput").ap()

    with tile.TileContext(nc) as tc:
        with tc.tile_pool(name="const", bufs=1) as cpool, \
             tc.tile_pool(name="pp", bufs=2) as pp, \
             tc.tile_pool(name="sp", bufs=2, space="PSUM") as sp, \
             tc.tile_pool(name="op", bufs=1, space="PSUM") as op, \
             tc.tile_pool(name="rp", bufs=1, space="PSUM") as rp, \
             tc.tile_pool(name="yp", bufs=1, space="PSUM") as yp:
            xT = cpool.tile([128, NCI, T], BF16)
            wq = cpool.tile([128, NCI, C], BF16)
            wk = cpool.tile([128, NCI, C], BF16)
            wv = cpool.tile([128, NCI, C], BF16)
            wp = cpool.tile([128, NCI, C], BF16)
            biasb = cpool.tile([128, C], F32)
            QT = cpool.tile([128, NCI, T], BF16)
            KT = cpool.tile([128, NCI, T], BF16)
            attT = cpool.tile([128, NCI, T], BF16)
            Vt = cpool.tile([128, 16, H, 65], BF16)
            ones16 = cpool.tile([1, 64], FP16)

            for ci in range(NCI):
                nc.sync.dma_start(xT[:, ci, :], xT_d[:, ci, :])
            nc.sync.dma_start(wq[:], wq_d[:])
            nc.sync.dma_start(wk[:], wk_d[:])
            nc.sync.dma_start(wv[:], wv_d[:])
            nc.sync.dma_start(wp[:], wp_d[:])
            nc.sync.dma_start(biasb[:], biasb_d[:])
            # whole-tile memset (strided memset fails ISA check); V copies
            # overwrite cols 0:64 leaving col 64 == 1.0 (denominator trick)
            nc.gpsimd.memset(Vt[:], 1.0)
            nc.gpsimd.memset(ones16[:], 1.0)

            # zero-init both S psum buffers: wide exp calls read full tiles
            # and must never see boot garbage (NaN) even in unused columns
            for _ in range(2):
                z0 = sp.tile([128, 1024], F32, tag="S")
                nc.vector.memset(z0[:], 0.0)

            # ---- phase-1 task closures (projections, drip-fed) ----
            def proj_qk(dst, w, bi, th):  # one [128,1024] column block
                ps = sp.tile([128, 1024], F32, tag="S")
                for sub in range(2):
                    for ci in range(NCI):
                        nc.tensor.matmul(
                            ps[:, 512 * sub:512 * sub + 512],
                            lhsT=w[:, ci, ts(bi, 128)],
                            rhs=xT[:, ci, ds(1024 * th + 512 * sub, 512)],
                            start=(ci == 0), stop=(ci == NCI - 1),
                        )
                nc.vector.tensor_copy(dst[:, bi, ds(1024 * th, 1024)], ps[:])

            def proj_v(si):
                ps = sp.tile([128, 1024], F32, tag="S")
                for ci in range(NCI):
                    nc.tensor.matmul(
                        ps[:, 0:C],
                        lhsT=xT[:, ci, ts(si, 128)],
                        rhs=wv[:, ci, :],
                        start=(ci == 0), stop=(ci == NCI - 1),
                    )
                nc.vector.tensor_copy(
                    Vt[:, si, :, 0:64],
                    ps[:, 0:C].rearrange("p (h d) -> p h d", h=H),
                )

            # prefix: exactly what (j=0, hp=0) needs
            for t in [lambda: proj_qk(KT, wk, 0, 0), lambda: proj_qk(QT, wq, 0, 0),
                      lambda: proj_v(0), lambda: proj_v(1), lambda: proj_v(2),
                      lambda: proj_v(3)]:
                t()
            drip = [
                lambda: proj_qk(KT, wk, 1, 0), lambda: proj_qk(QT, wq, 1, 0),
                lambda: proj_v(4), lambda: proj_v(5),
                lambda: proj_qk(KT, wk, 2, 0), lambda: proj_qk(QT, wq, 2, 0),
                lambda: proj_v(6), lambda: proj_v(7),
                lambda: proj_qk(KT, wk, 0, 1), lambda: proj_qk(QT, wq, 0, 1),
                lambda: proj_v(8), lambda: proj_v(9),
                lambda: proj_qk(KT, wk, 1, 1), lambda: proj_qk(QT, wq, 1, 1),
                lambda: proj_v(10), lambda: proj_v(11),
                lambda: proj_qk(KT, wk, 2, 1), lambda: proj_qk(QT, wq, 2, 1),
                lambda: proj_v(12), lambda: proj_v(13),
                lambda: proj_v(14), lambda: proj_v(15),
            ]
            drip_pos = 0
            ytasks = []  # deferred output-projection closures

            # ---- phase 2: attention ----
            for j in range(NJ):
                for hp in range(NHP):
                    ha, hb = 2 * hp, 2 * hp + 1
                    Oa = op.tile([65, TJ], F32, tag="Oa")
                    Ob = op.tile([65, TJ], F32, tag="Ob")
                    nch = 4 * j + 4
                    Ps = [None] * nch

                    def emit_S(i):
                        d = max(0, SC * i - TJ * j)
                        st = sp.tile([128, 1024], F32, tag="S")
                        for z in (0, 64):
                            nc.tensor.matmul(
                                st[:, 8 * z + d:8 * z + 512],
                                lhsT=KT[z:z + 64, hp, ts(i, SC)],
                                rhs=QT[z:z + 64, hp, ds(TJ * j + d, TJ - d)],
                                start=True, stop=True,
                            )
                        P = pp.tile([128, 1024], BF16, tag="P")
                        if d <= 128:
                            nc.scalar.activation(P[:], st[:], AF.Exp, scale=SCALE)
                        else:
                            for z in (0, 512):
                                nc.scalar.activation(P[:, z + d:z + 512],
                                                     st[:, z + d:z + 512],
                                                     AF.Exp, scale=SCALE)
                        if SC * i >= TJ * j:  # fringe: mask diag window
                            for z in (0, 512):
                                nc.gpsimd.affine_select(
                                    out=P[:, z + d:z + d + 128],
                                    in_=P[:, z + d:z + d + 128],
                                    pattern=[[1, 128]],
                                    compare_op=mybir.AluOpType.is_ge,
                                    fill=0.0, base=0, channel_multiplier=-1,
                                )
                        Ps[i] = P

                    def emit_PV(i):
                        d = max(0, SC * i - TJ * j)
                        for O, z, h in ((Oa, 0, ha), (Ob, 512, hb)):
                            nc.tensor.matmul(
                                O[:, d:TJ],
                                lhsT=Vt[:, i, h, :],
                                rhs=Ps[i][:, z + d:z + 512],
                                start=(i == 0), stop=(i == nch - 1),
                            )

                    for i in range(nch):
                        emit_S(i)
                        if i >= 1:
                            emit_PV(i - 1)
                        if i == 1:
                            # drip: 4 projection tasks per head-pair boundary
                            for _ in range(4):
                                if drip_pos < len(drip):
                                    drip[drip_pos]()
                                    drip_pos += 1
                        elif i >= 2 and ytasks:
                            ytasks.pop(0)()  # one deferred Y block per chunk
                    emit_PV(nch - 1)

                    # evict attT (unnormalized) + denominator rows
                    nc.vector.tensor_copy(attT[0:64, hp, ts(j, TJ)], Oa[0:64, :])
                    nc.vector.tensor_copy(attT[64:128, hp, ts(j, TJ)], Ob[0:64, :])
                    dsbA = pp.tile([1, TJ], F32, tag="dsbA")
                    dsbB = pp.tile([1, TJ], F32, tag="dsbB")
                    nc.vector.tensor_copy(dsbA[:], Oa[64:65, :])
                    nc.vector.tensor_copy(dsbB[:], Ob[64:65, :])
                    rrFa = pp.tile([1, TJ], F32, tag="rrFa")
                    rrFb = pp.tile([1, TJ], F32, tag="rrFb")
                    nc.vector.reciprocal_approx_fast(rrFa[:], dsbA[:])
                    nc.vector.reciprocal_approx_fast(rrFb[:], dsbB[:])
                    r16a = pp.tile([1, TJ], FP16, tag="r16a")
                    r16b = pp.tile([1, TJ], FP16, tag="r16b")
                    nc.vector.tensor_copy(r16a[:], rrFa[:])
                    nc.vector.tensor_copy(r16b[:], rrFb[:])
                    rB = rp.tile([128, TJ], F32, tag="rB")
                    for z, r16 in ((0, r16a), (64, r16b)):
                        for q in range(4):
                            nc.tensor.matmul(
                                rB[z:z + 64, ts(q, 128)],
                                lhsT=ones16[0:1, :],
                                rhs=r16[0:1, ts(q, 128)],
                                start=True, stop=True,
                            )
                    nc.vector.tensor_mul(attT[:, hp, ts(j, TJ)],
                                         attT[:, hp, ts(j, TJ)], rB[:])

                def make_y(j_, tb_):
                    def run():
                        Y = yp.tile([128, C], F32, tag="Y")
                        for bi in range(NCI):
                            nc.tensor.matmul(
                                Y[:],
                                lhsT=attT[:, bi, ts(4 * j_ + tb_, 128)],
                                rhs=wp[:, bi, :],
                                start=(bi == 0), stop=(bi == NCI - 1),
                            )
                        ysb = pp.tile([128, C], F32, tag="ysb")
                        nc.vector.tensor_add(ysb[:], Y[:], biasb[:])
                        nc.sync.dma_start(y_d[ts(4 * j_ + tb_, 128), :], ysb[:])
                    return run
                for tb in range(4):
                    ytasks.append(make_y(j, tb))

            while ytasks:  # tail: j=3 output blocks
                ytasks.pop(0)()

    nc.compile()
    return nc


def _prep_inputs(x, Wq, Wk, Wv, Wp, bp):
    """Host-side shard + layout prep. Returns per-core input maps."""
    bf = ml_dtypes.bfloat16
    x = np.asarray(x, dtype=np.float32)

    def pack_w(W):  # [H, C, Dh] -> [128, NCI, H*Dh]
        Whd = np.transpose(np.asarray(W, np.float32), (1, 0, 2)).reshape(C, H * DH)
        return np.ascontiguousarray(
            Whd.reshape(NCI, 128, H * DH).transpose(1, 0, 2)
        ).astype(bf)

    wq_p, wk_p, wv_p = pack_w(Wq), pack_w(Wk), pack_w(Wv)
    wp_p = np.ascontiguousarray(
        np.asarray(Wp, np.float32).reshape(NCI, 128, C).transpose(1, 0, 2)
    ).astype(bf)
    biasb = np.broadcast_to(np.asarray(bp, np.float32), (128, C)).copy()

    in_maps = []
    for b in range(B):
        xT = np.ascontiguousarray(
            x[b].T.reshape(NCI, 128, T).transpose(1, 0, 2)
        ).astype(bf)
        in_maps.append({
            "xT": xT, "wq": wq_p, "wk": wk_p, "wv": wv_p, "wp": wp_p,
            "biasb": biasb,
        })
    return in_maps


_CACHE = {}


def kernel(x, Wq, Wk, Wv, Wp, bp):
    from concourse.bass_utils import run_bass_kernel_spmd

    if "nc" not in _CACHE:
        _CACHE["nc"] = build_kernel()
    nc = _CACHE["nc"]
    in_maps = _prep_inputs(x, Wq, Wk, Wv, Wp, bp)
    res = run_bass_kernel_spmd(nc, in_maps, list(range(NCORES)))
    out = np.stack([res.results[b]["y"] for b in range(B)], axis=0)
    return out.astype(np.float32)


# revision 7
# speedup vs baseline: 1.4644x; 1.0764x over previous
"""Multi-head causal attention (B=8, T=2048, C=384, H=6, Dh=64) on 8 TRN2 cores.

Sharding: data-parallel over batch - core b computes batch element b end to end
(no collectives).

v3 layout (all "T" means transposed, head-dim/channel on partitions):
  xT   [128, 3, 2048]  bf16   c = 128*ci + p
  wq/wk[128, 3, 384]   bf16   packed Wq[h,c,d] -> [c, h*64+d]
  wv/wp[128, 3, 384]   bf16
  biasb[128, 384]      f32    bias replicated across partitions

Per-core compute:
  QT/KT [hd, t] via matmul; Vt [s, h, 65] augmented (col 64 == 1 -> denom row).
  Attention per (j q-block of 512, hp head-pair, i s-chunk of 128):
    S-pair [128, 1024] = two concurrent K=64 matmuls (tile rows 0:64 / 64:128)
    one wide exp (ACT) -> P bf16 [128, 1024]; causal diag via affine_select
    PV per head accumulates O[65, 512] (row 64 = softmax denominator)
  Denominator/normalize chain is DEFERRED into the next head-pair's chunk
  slots: denom rows [1,1024] -> reciprocal_approx_fast -> fp16 -> K=1
  broadcast matmuls -> recipB [128,512] psum -> tensor_mul normalizes attT in
  place.  Output projection is then a dense K=128 accumulation
  Y[t,e] = attT^T @ wp (+bias via tensor_add on eviction), one t-block per
  chunk slot.
  Phase-1 projection tasks share the recipB PSUM bank and are emitted
  need-driven (JIT) with a pre-pull hint order, so the PE stays dense while
  ACT streams exps.
"""

import numpy as np
import ml_dtypes

import concourse.bass as bass
import concourse.tile as tile
from concourse import bacc, mybir
from concourse.bass import ts, ds

F32 = mybir.dt.float32
BF16 = mybir.dt.bfloat16
FP16 = mybir.dt.float16
AF = mybir.ActivationFunctionType

B, T, C = 8, 2048, 384
H, DH = 6, 64
SCALE = DH ** -0.5
NCORES = 8
TJ = 512            # q-block width
NJ = T // TJ        # 4 q-blocks
SC = 128            # s-chunk
NCI = C // 128      # 3 channel chunks
NHP = H // 2        # 3 head pairs (= hd blocks)


def build_kernel():
    nc = bacc.Bacc("TRN2", target_bir_lowering=False, debug=False)

    xT_d = nc.dram_tensor("xT", [128, NCI, T], BF16, kind="ExternalInput").ap()
    wq_d = nc.dram_tensor("wq", [128, NCI, C], BF16, kind="ExternalInput").ap()
    wk_d = nc.dram_tensor("wk", [128, NCI, C], BF16, kind="ExternalInput").ap()
    wv_d = nc.dram_tensor("wv", [128, NCI, C], BF16, kind="ExternalInput").ap()
    wp_d = nc.dram_tensor("wp", [128, NCI, C], BF16, kind="ExternalInput").ap()
    biasb_d = nc.dram_tensor("biasb", [128, C], F32, kind="ExternalInput").ap()
    y_d = nc.dram_tensor("y", [T, C], F32, kind="ExternalOutput").ap()

    with tile.TileContext(nc) as tc:
        with tc.tile_pool(name="const", bufs=1) as cpool, \
             tc.tile_pool(name="pp", bufs=2) as pp, \
             tc.tile_pool(name="sp", bufs=2, space="PSUM") as sp, \
             tc.tile_pool(name="op", bufs=1, space="PSUM") as op, \
             tc.tile_pool(name="rp", bufs=1, space="PSUM") as rp, \
             tc.tile_pool(name="yp", bufs=1, space="PSUM") as yp:
            xT = cpool.tile([128, NCI, T], BF16)
            wq = cpool.tile([128, NCI, C], BF16)
            wk = cpool.tile([128, NCI, C], BF16)
            wv = cpool.tile([128, NCI, C], BF16)
            wp = cpool.tile([128, NCI, C], BF16)
            biasb = cpool.tile([128, C], F32)
            QT = cpool.tile([128, NCI, T], BF16)
            KT = cpool.tile([128, NCI, T], BF16)
            attT = cpool.tile([128, NCI, T], BF16)
            Vt = cpool.tile([128, 16, H, 65], BF16)
            ones16 = cpool.tile([1, 64], FP16)

            for ci in range(NCI):
                nc.sync.dma_start(xT[:, ci, :], xT_d[:, ci, :])
            nc.sync.dma_start(wq[:], wq_d[:])
            nc.sync.dma_start(wk[:], wk_d[:])
            nc.sync.dma_start(wv[:], wv_d[:])
            nc.sync.dma_start(wp[:], wp_d[:])
            nc.sync.dma_start(biasb[:], biasb_d[:])
            # whole-tile memset (strided memset fails ISA check); V copies
            # overwrite cols 0:64 leaving col 64 == 1.0 (denominator trick)
            nc.gpsimd.memset(Vt[:], 1.0)
            nc.gpsimd.memset(ones16[:], 1.0)

            # zero-init both S psum buffers: wide exp calls read full tiles
            # and must never see boot garbage (NaN) even in unused columns
            for _ in range(2):
                z0 = sp.tile([128, 1024], F32, tag="S")
                nc.vector.memset(z0[:], 0.0)

            # ---- phase-1 projection tasks ----
            # prefix (what j=0,hp=0 needs) runs on the S-tile rotation;
            # everything else shares the recipB psum bank, emitted JIT.
            def prefix_qk(dst, w, bi):
                ps = sp.tile([128, 1024], F32, tag="S")
                for sub in range(2):
                    for ci in range(NCI):
                        nc.tensor.matmul(
                            ps[:, 512 * sub:512 * sub + 512],
                            lhsT=w[:, ci, ts(bi, 128)],
                            rhs=xT[:, ci, ts(sub, 512)],
                            start=(ci == 0), stop=(ci == NCI - 1),
                        )
                nc.vector.tensor_copy(dst[:, bi, 0:1024], ps[:])

            def prefix_v(si):
                ps = sp.tile([128, 1024], F32, tag="S")
                for ci in range(NCI):
                    nc.tensor.matmul(
                        ps[:, 0:C],
                        lhsT=xT[:, ci, ts(si, 128)],
                        rhs=wv[:, ci, :],
                        start=(ci == 0), stop=(ci == NCI - 1),
                    )
                nc.vector.tensor_copy(
                    Vt[:, si, :, 0:64],
                    ps[:, 0:C].rearrange("p (h d) -> p h d", h=H),
                )

            done = set()

            def drip_qk(dst, w, bi, q):  # one 512-col quarter
                ps = rp.tile([128, TJ], F32, tag="W")
                for ci in range(NCI):
                    nc.tensor.matmul(
                        ps[:],
                        lhsT=w[:, ci, ts(bi, 128)],
                        rhs=xT[:, ci, ts(q, 512)],
                        start=(ci == 0), stop=(ci == NCI - 1),
                    )
                nc.vector.tensor_copy(dst[:, bi, ts(q, 512)], ps[:])

            def drip_v(si):
                ps = rp.tile([128, TJ], F32, tag="W")
                for ci in range(NCI):
                    nc.tensor.matmul(
                        ps[:, 0:C],
                        lhsT=xT[:, ci, ts(si, 128)],
                        rhs=wv[:, ci, :],
                        start=(ci == 0), stop=(ci == NCI - 1),
                    )
                nc.vector.tensor_copy(
                    Vt[:, si, :, 0:64],
                    ps[:, 0:C].rearrange("p (h d) -> p h d", h=H),
                )

            def make_task(key):
                kind = key[0]
                if kind == 'K':
                    return lambda: drip_qk(KT, wk, key[1], key[2])
                if kind == 'Q':
                    return lambda: drip_qk(QT, wq, key[1], key[2])
                return lambda: drip_v(key[1])

            def emit_key(key):
                if key not in done:
                    done.add(key)
                    make_task(key)()

            # prefix: (j=0, hp=0) needs K/Q block 0 cols 0:1024, V chunks 0-3
            prefix_qk(KT, wk, 0)
            prefix_qk(QT, wq, 0)
            for si in range(4):
                prefix_v(si)
            for q in range(2):
                done.add(('K', 0, q))
                done.add(('Q', 0, q))
            for si in range(4):
                done.add(('V', si))

            hint = [
                ('K', 1, 0), ('Q', 1, 0), ('K', 2, 0), ('Q', 2, 0),
                ('Q', 0, 1), ('K', 0, 1), ('Q', 1, 1), ('V', 4),
                ('V', 5), ('V', 6), ('V', 7), ('Q', 2, 1),
                ('K', 1, 1), ('V', 8), ('V', 9), ('K', 2, 1),
                ('Q', 0, 2), ('K', 0, 2), ('V', 10), ('V', 11),
                ('Q', 1, 2), ('K', 1, 2), ('Q', 2, 2), ('Q', 0, 3),
                ('K', 2, 2), ('K', 0, 3), ('Q', 1, 3), ('K', 1, 3),
                ('Q', 2, 3), ('V', 12), ('V', 13), ('V', 14),
                ('V', 15), ('K', 2, 3),
            ]
            hint_pos = 0

            def pull_hint():
                nonlocal hint_pos
                while hint_pos < len(hint):
                    key = hint[hint_pos]
                    hint_pos += 1
                    if key not in done:
                        emit_key(key)
                        return True
                return False

            ytasks = []      # deferred output-projection closures
            deferred = []    # deferred normalize chain (one per finished hp)

            # ---- phase 2: attention ----
            for j in range(NJ):
                for hp in range(NHP):
                    ha, hb = 2 * hp, 2 * hp + 1
                    # JIT guarantee: this frame's K quarters / Q block
                    for q in range(j + 1):
                        emit_key(('K', hp, q))
                    emit_key(('Q', hp, j))
                    Oa = op.tile([65, TJ], F32, tag="Oa")
                    Ob = op.tile([65, TJ], F32, tag="Ob")
                    nch = 4 * j + 4
                    Ps = [None] * nch

                    def emit_S(i):
                        d = max(0, SC * i - TJ * j)
                        st = sp.tile([128, 1024], F32, tag="S")
                        for z in (0, 64):
                            nc.tensor.matmul(
                                st[:, 8 * z + d:8 * z + 512],
                                lhsT=KT[z:z + 64, hp, ts(i, SC)],
                                rhs=QT[z:z + 64, hp, ds(TJ * j + d, TJ - d)],
                                start=True, stop=True,
                            )
                        P = pp.tile([128, 1024], BF16, tag="P")
                        if d <= 128:
                            nc.scalar.activation(P[:], st[:], AF.Exp, scale=SCALE)
                        else:
                            for z in (0, 512):
                                nc.scalar.activation(P[:, z + d:z + 512],
                                                     st[:, z + d:z + 512],
                                                     AF.Exp, scale=SCALE)
                        if SC * i >= TJ * j:  # fringe: mask diag window
                            for z in (0, 512):
                                nc.gpsimd.affine_select(
                                    out=P[:, z + d:z + d + 128],
                                    in_=P[:, z + d:z + d + 128],
                                    pattern=[[1, 128]],
                                    compare_op=mybir.AluOpType.is_ge,
                                    fill=0.0, base=0, channel_multiplier=-1,
                                )
                        Ps[i] = P

                    def emit_PV(i):
                        d = max(0, SC * i - TJ * j)
                        emit_key(('V', i))
                        for O, z, h in ((Oa, 0, ha), (Ob, 512, hb)):
                            nc.tensor.matmul(
                                O[:, d:TJ],
                                lhsT=Vt[:, i, h, :],
                                rhs=Ps[i][:, z + d:z + 512],
                                start=(i == 0), stop=(i == nch - 1),
                            )

                    norm_slot = 3 if nch == 4 else 5
                    for i in range(nch):
                        emit_S(i)
                        if i >= 1:
                            emit_PV(i - 1)
                        if i in (1, 2):
                            pull_hint()
                        elif i == norm_slot:
                            if deferred:
                                deferred.pop(0)()
                        elif i >= 4:
                            if not pull_hint() and ytasks and i >= 6:
                                ytasks.pop(0)()
                    emit_PV(nch - 1)

                    # evict attT (unnormalized) + denominator rows; the rest
                    # of the normalize chain is deferred into the next frame
                    nc.vector.tensor_copy(attT[0:64, hp, ts(j, TJ)], Oa[0:64, :])
                    nc.vector.tensor_copy(attT[64:128, hp, ts(j, TJ)], Ob[0:64, :])
                    dsb = pp.tile([1, 2 * TJ], F32, tag="dsb")
                    nc.vector.tensor_copy(dsb[0:1, 0:TJ], Oa[64:65, :])
                    nc.vector.tensor_copy(dsb[0:1, TJ:2 * TJ], Ob[64:65, :])

                    def make_norm(j_, hp_, dsb_):
                        def run():
                            rr = pp.tile([1, 2 * TJ], F32, tag="rr")
                            nc.vector.reciprocal_approx_fast(rr[:], dsb_[:])
                            r16 = pp.tile([1, 2 * TJ], FP16, tag="r16")
                            nc.vector.tensor_copy(r16[:], rr[:])
                            rB = rp.tile([128, TJ], F32, tag="W")
                            for z in (0, 64):
                                for q in range(4):
                                    nc.tensor.matmul(
                                        rB[z:z + 64, ts(q, 128)],
                                        lhsT=ones16[0:1, :],
                                        rhs=r16[0:1, ds(8 * z + 128 * q, 128)],
                                        start=True, stop=True,
                                    )
                            nc.vector.tensor_mul(attT[:, hp_, ts(j_, TJ)],
                                                 attT[:, hp_, ts(j_, TJ)],
                                                 rB[:, 0:TJ])
                        return run
                    deferred.append(make_norm(j, hp, dsb))

                def make_y(j_, tb_):
                    def run():
                        Y = yp.tile([128, C], F32, tag="Y")
                        for bi in range(NCI):
                            nc.tensor.matmul(
                                Y[:],
                                lhsT=attT[:, bi, ts(4 * j_ + tb_, 128)],
                                rhs=wp[:, bi, :],
                                start=(bi == 0), stop=(bi == NCI - 1),
                            )
                        ysb = pp.tile([128, C], F32, tag="ysb")
                        nc.vector.tensor_add(ysb[:], Y[:], biasb[:])
                        nc.sync.dma_start(y_d[ts(4 * j_ + tb_, 128), :], ysb[:])
                    return run
                for tb in range(4):
                    ytasks.append(make_y(j, tb))

            while deferred:  # tail: last normalize + j=3 output blocks
                deferred.pop(0)()
            while ytasks:
                ytasks.pop(0)()

    nc.compile()
    return nc


def _prep_inputs(x, Wq, Wk, Wv, Wp, bp):
    """Host-side shard + layout prep. Returns per-core input maps."""
    bf = ml_dtypes.bfloat16
    x = np.asarray(x, dtype=np.float32)

    def pack_w(W):  # [H, C, Dh] -> [128, NCI, H*Dh]
        Whd = np.transpose(np.asarray(W, np.float32), (1, 0, 2)).reshape(C, H * DH)
        return np.ascontiguousarray(
            Whd.reshape(NCI, 128, H * DH).transpose(1, 0, 2)
        ).astype(bf)

    wq_p, wk_p, wv_p = pack_w(Wq), pack_w(Wk), pack_w(Wv)
    wp_p = np.ascontiguousarray(
        np.asarray(Wp, np.float32).reshape(NCI, 128, C).transpose(1, 0, 2)
    ).astype(bf)
    biasb = np.broadcast_to(np.asarray(bp, np.float32), (128, C)).copy()

    in_maps = []
    for b in range(B):
        xT = np.ascontiguousarray(
            x[b].T.reshape(NCI, 128, T).transpose(1, 0, 2)
        ).astype(bf)
        in_maps.append({
            "xT": xT, "wq": wq_p, "wk": wk_p, "wv": wv_p, "wp": wp_p,
            "biasb": biasb,
        })
    return in_maps


_CACHE = {}


def kernel(x, Wq, Wk, Wv, Wp, bp):
    from concourse.bass_utils import run_bass_kernel_spmd

    if "nc" not in _CACHE:
        _CACHE["nc"] = build_kernel()
    nc = _CACHE["nc"]
    in_maps = _prep_inputs(x, Wq, Wk, Wv, Wp, bp)
    res = run_bass_kernel_spmd(nc, in_maps, list(range(NCORES)))
    out = np.stack([res.results[b]["y"] for b in range(B)], axis=0)
    return out.astype(np.float32)
